# revision 6
# baseline (speedup 1.0000x reference)
"""Graph Wavelet Neural Network forward pass on 8 Trainium2 NeuronCores.

Computation: out = wavelets @ diag(filt) @ wavelets_inv @ features @ W
  N=8192, C_IN=256, C_OUT=128.

Strategy (memory regime: streaming the two [8192,8192] matrices dominates):
  - Core j owns row-block jb of wavelets_inv (-> right rows jb) and
    column-block jb of wavelets (-> full-shape partial of out; host sums
    the 8 partials). No device collectives.
  - Operands are pre-transposed/pre-blocked on the host so the contraction
    index lands on SBUF partitions and EVERY device DMA is one fully
    contiguous block:
      ft     = features.T                  [256, 8192]   (replicated)
      winv_t = (filt * wavelets_inv)[jb].T  [8192, 1024]  (per-core)
      wav_b  = wavelets[:, jb].T chunk-major [8*1024, 1024] (per-core)
    filt is folded into wavelets_inv rows on the host (free O(N^2)).
  - Big streams are bf16: halves HBM traffic (the roofline) and runs the
    PE at 1 cycle/row. PSUM accumulation stays fp32. Output partials are
    bf16, chunk-major so writes are contiguous.
  - Device pipeline (core j):
      T    = features @ W     (host), DMA'd once as 64 k-tiles
      SR^T = sum_k T[k].T @ winv_t[k]   [128, 1024] psum accumulation
      SR   = PE-transpose(SR^T)         8 tiles [128m, 128c]
      o^T  = sum_m SR[m].T @ wav[m, nch]  per 1024-wide n-chunk
  - TRN2 PE p-state: the clock ramps 0.65->1.2->2.4 GHz only under
    SUSTAINED execution; any idle gap resets the ramp. At 1.2 GHz the PE
    consumes bf16 slower than the DMA delivers, so a DMA-paced schedule
    locks the clock low and the PE then paces the DMA down (~305 GB/s
    observed). Fix: zero "filler" matmuls into a scratch PSUM bank are
    interleaved so the PE never idles, holding the 2.4 GHz state; real
    work then always clears faster than the stream.
  - DMA topology: 2MB transfers (fewer sync edges -> shorter semaphore
    teardown). wi/wv groups alternate the scalar/sync HWDGE rings; the
    T preload and all output flushes ride the vector ring so result
    writes never head-of-line block the input streams.
"""

import os

import numpy as np

import concourse.bass as bass
import concourse.mybir as mybir
import concourse.tile as tile
from concourse import bacc
from concourse.bass_utils import run_bass_kernel_spmd

N = 8192
C_IN = 256
C_OUT = 128
M = 8  # cores
B = N // M  # 1024 rows per core
KT = N // 128  # 64 contraction tiles
MT = B // 128  # 8 row tiles per core block
NCH = 1024  # output free-dim chunk
NC = N // NCH  # 8 chunks
GK = 8  # k-tiles per 2MB wi group
NG = KT // GK  # 8 wi groups
F32 = mybir.dt.float32

# PE keepalive filler counts (tuned against the perfetto trace)
HEAD_FILL = 5  # while the first wi sub-group + T head stream in
FILL_B0 = 2  # between first-group sub-DMA pairs
FILL_B = 3  # per stage-B half-group (1MB)
FILL_D = 3  # per stage-D half-chunk (1MB)

_cache = {}


def _build():
    SDT = mybir.dt.bfloat16
    nc = bacc.Bacc("TRN2", target_bir_lowering=False, debug=False)
    t_d = nc.dram_tensor("t_d", [N, C_OUT], SDT, kind="ExternalInput")
    winv_t = nc.dram_tensor("winv_t", [N, B], SDT, kind="ExternalInput")
    wav_b = nc.dram_tensor("wav_b", [NC * B, NCH], SDT, kind="ExternalInput")
    ident_d = nc.dram_tensor("ident", [128, 128], SDT, kind="ExternalInput")
    outp = nc.dram_tensor("outp", [NC * C_OUT, NCH], SDT, kind="ExternalOutput")

    with tile.TileContext(nc) as tc:
        with (
            tc.tile_pool(name="const", bufs=1) as cpool,
            tc.tile_pool(name="stream", bufs=4) as spool,
            tc.tile_pool(name="opool", bufs=3) as opool,
            tc.tile_pool(name="ps_a", bufs=2, space="PSUM") as ps_a,
            tc.tile_pool(name="ps_r", bufs=1, space="PSUM") as ps_r,
            tc.tile_pool(name="ps_o", bufs=2, space="PSUM") as ps_o,
        ):
            # --- constants ---
            fillsrc = cpool.tile([128, 512], SDT, tag="fillsrc")
            nc.gpsimd.memset(fillsrc, 0.0)
            ident = cpool.tile([128, 128], SDT, tag="ident")
            nc.gpsimd.dma_start(out=ident, in_=ident_d.ap())

            def filler(n):
                # zero matmuls; keep the PE p-state ramped between real groups
                for _ in range(n):
                    pf = ps_a.tile([128, 512], F32, tag="psA")
                    nc.tensor.matmul(
                        pf, fillsrc[:, 0:128], fillsrc, start=True, stop=True
                    )

            # --- head DMAs: T preload on the gpsimd ring, split so the
            # first k-tiles land fast; first wi group split into 512KB
            # sub-DMAs so real matmuls start ~3us in. PE runs fillers
            # until the data lands.
            t_sb = cpool.tile([128, KT * 128], SDT, tag="tsb")
            t_src = t_d.ap().rearrange("(p g) f -> p g f", g=KT)
            t_dst = t_sb.rearrange("p (g f) -> p g f", g=KT)
            nc.gpsimd.dma_start(out=t_dst[:, 0:GK, :], in_=t_src[:, 0:GK, :])
            # t k-tiles 8..63 are DMA'd later, gated by WAR deps on fillers
            # placed inside group 0 below, so the head bandwidth goes to
            # t0+wi0+wi1 first.
            filler(HEAD_FILL)

            # --- stage B: SR^T accumulation over 8 groups x 8 k-tiles ---
            ps_sr = ps_r.tile([128, B], F32, tag="psR")
            for g in range(NG):
                wi = spool.tile([128, GK * B], SDT, tag="wi", bufs=3)
                eng = nc.scalar if g % 2 == 0 else nc.sync
                wi_dst = wi.rearrange("p (a f) -> p a f", a=GK)
                wi_src = winv_t.ap()[
                    g * GK * 128 : (g + 1) * GK * 128, :
                ].rearrange("(p a) f -> p a f", a=GK)
                if g == 0:
                    for sbl in range(4):
                        eng.dma_start(
                            out=wi_dst[:, 2 * sbl : 2 * sbl + 2, :],
                            in_=wi_src[:, 2 * sbl : 2 * sbl + 2, :],
                        )
                else:
                    # 1MB halves: finer deps smooth PE data-arrival jitter
                    eng.dma_start(
                        out=wi_dst[:, 0 : GK // 2, :], in_=wi_src[:, 0 : GK // 2, :]
                    )
                    eng.dma_start(
                        out=wi_dst[:, GK // 2 : GK, :],
                        in_=wi_src[:, GK // 2 : GK, :],
                    )

                def mm_b(a, h):
                    k = GK * g + a
                    nc.tensor.matmul(
                        ps_sr[:, h * 512 : (h + 1) * 512],
                        t_sb[:, k * 128 : (k + 1) * 128],
                        wi[:, a * B + h * 512 : a * B + (h + 1) * 512],
                        start=(k == 0),
                        stop=(k == KT - 1),
                    )

                if g < NG - 1:
                    for a in range(GK):
                        for h in range(2):
                            mm_b(a, h)
                        if g == 0 and a % 2 == 1 and a < GK - 1:
                            filler(FILL_B0)
                        if g == 0 and a == 3:
                            # WAR gate: reading t[8:16] forces the t1 DMA
                            # to wait until group 0 is half-consumed
                            pf = ps_a.tile([128, 512], F32, tag="psA")
                            nc.tensor.matmul(
                                pf,
                                fillsrc[:, 0:128],
                                t_sb[:, GK * 128 : GK * 128 + 512],
                                start=True,
                                stop=True,
                            )
                            nc.gpsimd.dma_start(
                                out=t_dst[:, GK : 2 * GK, :],
                                in_=t_src[:, GK : 2 * GK, :],
                            )
                        if g == 0 and a == 6:
                            pf = ps_a.tile([128, 512], F32, tag="psA")
                            nc.tensor.matmul(
                                pf,
                                fillsrc[:, 0:128],
                                t_sb[:, 2 * GK * 128 : 2 * GK * 128 + 512],
                                start=True,
                                stop=True,
                            )
                            nc.gpsimd.dma_start(
                                out=t_dst[:, 2 * GK : KT, :],
                                in_=t_src[:, 2 * GK : KT, :],
                            )
                        if a == GK // 2 - 1:
                            filler(FILL_B)
                    filler(FILL_B)
                else:
                    # h-outer: half 0 of SR^T finalizes early so its cast
                    # and transposes overlap half 1's matmuls
                    for h in range(2):
                        for a in range(GK):
                            mm_b(a, h)

            srT = cpool.tile([128, B], SDT, tag="srT")
            nc.vector.tensor_copy(srT[:, 0:512], ps_sr[:, 0:512])
            nc.vector.tensor_copy(srT[:, 512:B], ps_sr[:, 512:B])

            # --- stage C: SR tiles = transpose(SR^T) ---
            sr_all = cpool.tile([128, B], SDT, tag="srall")
            for mt in range(MT):
                pst = ps_a.tile([128, 128], SDT, tag="psA")
                nc.tensor.transpose(pst, srT[:, mt * 128 : (mt + 1) * 128], ident)
                nc.vector.tensor_copy(sr_all[:, mt * 128 : (mt + 1) * 128], pst)

            # --- stage D: out^T partial chunks ---
            for c in range(NC):
                wv = spool.tile([128, MT * NCH], SDT, tag="wv", bufs=6)
                eng = nc.scalar if c % 2 == 0 else nc.sync
                wv_dst = wv.rearrange("p (a f) -> p a f", a=MT)
                wv_src = wav_b.ap()[c * B : (c + 1) * B, :].rearrange(
                    "(p a) f -> p a f", a=MT
                )
                eng.dma_start(out=wv_dst[:, 0 : MT // 2, :], in_=wv_src[:, 0 : MT // 2, :])
                eng.dma_start(out=wv_dst[:, MT // 2 : MT, :], in_=wv_src[:, MT // 2 : MT, :])
                ps_out = ps_o.tile([128, NCH], F32, tag="psO")
                last = c == NC - 1
                if not last:
                    for mt in range(MT):
                        lhs = sr_all[:, mt * 128 : (mt + 1) * 128]
                        for h in range(2):
                            nc.tensor.matmul(
                                ps_out[:, h * 512 : (h + 1) * 512],
                                lhs,
                                wv[:, mt * NCH + h * 512 : mt * NCH + (h + 1) * 512],
                                start=(mt == 0),
                                stop=(mt == MT - 1),
                            )
                        if mt == MT // 2 - 1:
                            filler(FILL_D)
                    filler(FILL_D)
                    ot = opool.tile([128, NCH], SDT, tag="ot", bufs=3)
                    nc.vector.tensor_copy(ot, ps_out)
                    nc.gpsimd.dma_start(
                        out=outp.ap()[c * C_OUT : (c + 1) * C_OUT, :], in_=ot
                    )
                else:
                    # h-outer on the final chunk: the first half's cast+flush
                    # overlaps the second half's matmuls, shortening the tail
                    for h in range(2):
                        for mt in range(MT):
                            lhs = sr_all[:, mt * 128 : (mt + 1) * 128]
                            nc.tensor.matmul(
                                ps_out[:, h * 512 : (h + 1) * 512],
                                lhs,
                                wv[:, mt * NCH + h * 512 : mt * NCH + (h + 1) * 512],
                                start=(mt == 0),
                                stop=(mt == MT - 1),
                            )
                        ot = opool.tile([128, 512], SDT, tag="oth", bufs=2)
                        nc.vector.tensor_copy(ot, ps_out[:, h * 512 : (h + 1) * 512])
                        nc.gpsimd.dma_start(
                            out=outp.ap()[
                                c * C_OUT : (c + 1) * C_OUT,
                                h * 512 : (h + 1) * 512,
                            ],
                            in_=ot,
                        )
    nc.compile()
    return nc


def make_in_maps(features, wavelets, wavelets_inv, weight_matrix, filt):
    import ml_dtypes

    sdt = ml_dtypes.bfloat16
    features = np.ascontiguousarray(features, dtype=np.float32)
    wavelets = np.ascontiguousarray(wavelets, dtype=np.float32)
    wavelets_inv = np.ascontiguousarray(wavelets_inv, dtype=np.float32)
    weight_matrix = np.ascontiguousarray(weight_matrix, dtype=np.float32)
    filt = np.ascontiguousarray(filt, dtype=np.float32)

    t_full = features @ weight_matrix
    # row p*KT+g holds T[g*128+p]: per-partition-contiguous for 16KB descriptors
    t_host = np.ascontiguousarray(
        t_full.reshape(KT, 128, C_OUT).transpose(1, 0, 2).reshape(N, C_OUT)
    ).astype(sdt)
    in_maps = []
    for j in range(M):
        jb = slice(j * B, (j + 1) * B)
        wt = (wavelets_inv[jb, :] * filt[jb, None]).T  # [N, B]
        # within each 2MB group: row p*GK+a holds wt[g*1024 + a*128 + p]
        winv_t = np.ascontiguousarray(
            wt.reshape(NG, GK, 128, B).transpose(0, 2, 1, 3).reshape(N, B)
        ).astype(sdt)
        # chunk-major blocking of wavelets[:, jb].T: row c*B + m
        wav_t = wavelets[:, jb].T  # [B, N]
        # chunk-major; within a chunk: row p*MT+mt holds wav_t[mt*128+p]
        wav_b = np.ascontiguousarray(
            wav_t.reshape(MT, 128, NC, NCH).transpose(2, 1, 0, 3).reshape(NC * B, NCH)
        ).astype(sdt)
        in_maps.append(
            {
                "t_d": t_host,
                "winv_t": winv_t,
                "wav_b": wav_b,
                "ident": np.eye(128, dtype=np.float32).astype(sdt),
            }
        )
    return in_maps


def combine_outputs(results):
    acc = results[0]["outp"].astype(np.float64)
    for j in range(1, M):
        acc += results[j]["outp"]
    # outp rows are [c][cc]: row c*C_OUT + cc holds out^T[cc, c*NCH:...]
    out_t = acc.reshape(NC, C_OUT, NCH).transpose(1, 0, 2).reshape(C_OUT, N)
    return np.ascontiguousarray(out_t.T.astype(np.float32))


def kernel(features, wavelets, wavelets_inv, weight_matrix, filt):
    os.environ.setdefault("BASS_NEVER_TRACE", "1")
    if "nc" not in _cache:
        _cache["nc"] = _build()
    nc = _cache["nc"]
    in_maps = make_in_maps(features, wavelets, wavelets_inv, weight_matrix, filt)
    res = run_bass_kernel_spmd(nc, in_maps, core_ids=list(range(M)))
    return combine_outputs(res.results)


# revision 7
# speedup vs baseline: 1.0359x; 1.0359x over previous
"""Graph Wavelet Neural Network forward pass on 8 Trainium2 NeuronCores.

Computation: out = wavelets @ diag(filt) @ wavelets_inv @ features @ W
  N=8192, C_IN=256, C_OUT=128.

Strategy (memory regime: streaming the two [8192,8192] matrices dominates):
  - Core j owns row-block jb of wavelets_inv (-> right rows jb) and
    column-block jb of wavelets (-> full-shape partial of out; host sums
    the 8 partials). No device collectives.
  - Operands are pre-transposed/pre-blocked on the host so the contraction
    index lands on SBUF partitions and EVERY device DMA is one fully
    contiguous block:
      ft     = features.T                  [256, 8192]   (replicated)
      winv_t = (filt * wavelets_inv)[jb].T  [8192, 1024]  (per-core)
      wav_b  = wavelets[:, jb].T chunk-major [8*1024, 1024] (per-core)
    filt is folded into wavelets_inv rows on the host (free O(N^2)).
  - Big streams are bf16: halves HBM traffic (the roofline) and runs the
    PE at 1 cycle/row. PSUM accumulation stays fp32. Output partials are
    bf16, chunk-major so writes are contiguous.
  - Device pipeline (core j):
      T    = features @ W     (host), DMA'd once as 64 k-tiles
      SR^T = sum_k T[k].T @ winv_t[k]   [128, 1024] psum accumulation
      SR   = PE-transpose(SR^T)         8 tiles [128m, 128c]
      o^T  = sum_m SR[m].T @ wav[m, nch]  per 1024-wide n-chunk
  - TRN2 PE p-state: the clock ramps 0.65->1.2->2.4 GHz only under
    SUSTAINED execution; any idle gap resets the ramp. At 1.2 GHz the PE
    consumes bf16 slower than the DMA delivers, so a DMA-paced schedule
    locks the clock low and the PE then paces the DMA down (~305 GB/s
    observed). Fix: zero "filler" matmuls into a scratch PSUM bank are
    interleaved so the PE never idles, holding the 2.4 GHz state; real
    work then always clears faster than the stream.
  - DMA topology: 2MB transfers (fewer sync edges -> shorter semaphore
    teardown). wi/wv groups alternate the scalar/sync HWDGE rings; the
    T preload and all output flushes ride the vector ring so result
    writes never head-of-line block the input streams.
"""

import os

import numpy as np

import concourse.bass as bass
import concourse.mybir as mybir
import concourse.tile as tile
from concourse import bacc
from concourse.bass_utils import run_bass_kernel_spmd

N = 8192
C_IN = 256
C_OUT = 128
M = 8  # cores
B = N // M  # 1024 rows per core
KT = N // 128  # 64 contraction tiles
MT = B // 128  # 8 row tiles per core block
NCH = 1024  # output free-dim chunk
NC = N // NCH  # 8 chunks
GK = 8  # k-tiles per 2MB wi group
NG = KT // GK  # 8 wi groups
F32 = mybir.dt.float32

# PE keepalive filler counts (tuned against the perfetto trace)
HEAD_FILL = 5  # while the first wi sub-group + T head stream in
FILL_B0 = 2  # between first-group sub-DMA pairs
FILL_B = 5  # per stage-B half-group (1MB)
FILL_D = 5  # per stage-D half-chunk (1MB)

_cache = {}


def _build():
    SDT = mybir.dt.bfloat16
    nc = bacc.Bacc("TRN2", target_bir_lowering=False, debug=False)
    t_d = nc.dram_tensor("t_d", [N, C_OUT], SDT, kind="ExternalInput")
    winv_t = nc.dram_tensor("winv_t", [N, B], SDT, kind="ExternalInput")
    wav_b = nc.dram_tensor("wav_b", [NC * B, NCH], SDT, kind="ExternalInput")
    ident_d = nc.dram_tensor("ident", [128, 128], SDT, kind="ExternalInput")
    outp = nc.dram_tensor("outp", [NC * C_OUT, NCH], SDT, kind="ExternalOutput")

    with tile.TileContext(nc) as tc:
        with (
            tc.tile_pool(name="const", bufs=1) as cpool,
            tc.tile_pool(name="stream", bufs=4) as spool,
            tc.tile_pool(name="opool", bufs=3) as opool,
            tc.tile_pool(name="ps_a", bufs=2, space="PSUM") as ps_a,
            tc.tile_pool(name="ps_r", bufs=1, space="PSUM") as ps_r,
            tc.tile_pool(name="ps_o", bufs=2, space="PSUM") as ps_o,
        ):
            # --- constants ---
            fillsrc = cpool.tile([128, 512], SDT, tag="fillsrc")
            nc.gpsimd.memset(fillsrc, 0.0)
            ident = cpool.tile([128, 128], SDT, tag="ident")
            nc.gpsimd.dma_start(out=ident, in_=ident_d.ap())

            def filler(n):
                # zero matmuls; keep the PE p-state ramped between real groups
                for _ in range(n):
                    pf = ps_a.tile([128, 512], F32, tag="psA")
                    nc.tensor.matmul(
                        pf, fillsrc[:, 0:128], fillsrc, start=True, stop=True
                    )

            # --- head DMAs: T preload on the gpsimd ring, split so the
            # first k-tiles land fast; first wi group split into 512KB
            # sub-DMAs so real matmuls start ~3us in. PE runs fillers
            # until the data lands.
            t_sb = cpool.tile([128, KT * 128], SDT, tag="tsb")
            t_src = t_d.ap().rearrange("(p g) f -> p g f", g=KT)
            t_dst = t_sb.rearrange("p (g f) -> p g f", g=KT)
            nc.gpsimd.dma_start(out=t_dst[:, 0:GK, :], in_=t_src[:, 0:GK, :])
            # t k-tiles 8..63 are DMA'd later, gated by WAR deps on fillers
            # placed inside group 0 below, so the head bandwidth goes to
            # t0+wi0+wi1 first.
            filler(HEAD_FILL)

            # --- stage B: SR^T accumulation over 8 groups x 8 k-tiles ---
            ps_sr = ps_r.tile([128, B], F32, tag="psR")
            for g in range(NG):
                wi = spool.tile([128, GK * B], SDT, tag="wi", bufs=4)
                eng = nc.scalar if g % 2 == 0 else nc.sync
                wi_dst = wi.rearrange("p (a f) -> p a f", a=GK)
                wi_src = winv_t.ap()[
                    g * GK * 128 : (g + 1) * GK * 128, :
                ].rearrange("(p a) f -> p a f", a=GK)
                if g == 0:
                    for sbl in range(4):
                        eng.dma_start(
                            out=wi_dst[:, 2 * sbl : 2 * sbl + 2, :],
                            in_=wi_src[:, 2 * sbl : 2 * sbl + 2, :],
                        )
                else:
                    # 1MB halves: finer deps smooth PE data-arrival jitter
                    eng.dma_start(
                        out=wi_dst[:, 0 : GK // 2, :], in_=wi_src[:, 0 : GK // 2, :]
                    )
                    eng.dma_start(
                        out=wi_dst[:, GK // 2 : GK, :],
                        in_=wi_src[:, GK // 2 : GK, :],
                    )

                def mm_b(a, h):
                    k = GK * g + a
                    nc.tensor.matmul(
                        ps_sr[:, h * 512 : (h + 1) * 512],
                        t_sb[:, k * 128 : (k + 1) * 128],
                        wi[:, a * B + h * 512 : a * B + (h + 1) * 512],
                        start=(k == 0),
                        stop=(k == KT - 1),
                    )

                if g < NG - 1:
                    for a in range(GK):
                        for h in range(2):
                            mm_b(a, h)
                        if g == 0 and a % 2 == 1 and a < GK - 1:
                            filler(FILL_B0)
                        if g == 0 and a == 3:
                            # WAR gate: reading t[8:16] forces the t1 DMA
                            # to wait until group 0 is half-consumed
                            pf = ps_a.tile([128, 512], F32, tag="psA")
                            nc.tensor.matmul(
                                pf,
                                fillsrc[:, 0:128],
                                t_sb[:, GK * 128 : GK * 128 + 512],
                                start=True,
                                stop=True,
                            )
                            nc.gpsimd.dma_start(
                                out=t_dst[:, GK : 2 * GK, :],
                                in_=t_src[:, GK : 2 * GK, :],
                            )
                        if g == 0 and a == 6:
                            pf = ps_a.tile([128, 512], F32, tag="psA")
                            nc.tensor.matmul(
                                pf,
                                fillsrc[:, 0:128],
                                t_sb[:, 2 * GK * 128 : 2 * GK * 128 + 512],
                                start=True,
                                stop=True,
                            )
                            nc.gpsimd.dma_start(
                                out=t_dst[:, 2 * GK : KT, :],
                                in_=t_src[:, 2 * GK : KT, :],
                            )
                        if a == GK // 2 - 1:
                            filler(FILL_B)
                    filler(FILL_B)
                else:
                    # h-outer: half 0 of SR^T finalizes early so its cast
                    # and transposes overlap half 1's matmuls
                    for h in range(2):
                        for a in range(GK):
                            mm_b(a, h)

            srT = cpool.tile([128, B], SDT, tag="srT")
            nc.vector.tensor_copy(srT[:, 0:512], ps_sr[:, 0:512])
            nc.vector.tensor_copy(srT[:, 512:B], ps_sr[:, 512:B])

            # --- stage C: SR tiles = transpose(SR^T) ---
            sr_all = cpool.tile([128, B], SDT, tag="srall")
            for mt in range(MT):
                pst = ps_a.tile([128, 128], SDT, tag="psA")
                nc.tensor.transpose(pst, srT[:, mt * 128 : (mt + 1) * 128], ident)
                nc.vector.tensor_copy(sr_all[:, mt * 128 : (mt + 1) * 128], pst)

            # --- stage D: out^T partial chunks ---
            for c in range(NC):
                wv = spool.tile([128, MT * NCH], SDT, tag="wv", bufs=6)
                eng = nc.scalar if c % 2 == 0 else nc.sync
                wv_dst = wv.rearrange("p (a f) -> p a f", a=MT)
                wv_src = wav_b.ap()[c * B : (c + 1) * B, :].rearrange(
                    "(p a) f -> p a f", a=MT
                )
                eng.dma_start(out=wv_dst[:, 0 : MT // 2, :], in_=wv_src[:, 0 : MT // 2, :])
                eng.dma_start(out=wv_dst[:, MT // 2 : MT, :], in_=wv_src[:, MT // 2 : MT, :])
                ps_out = ps_o.tile([128, NCH], F32, tag="psO")
                last = c == NC - 1
                if not last:
                    for mt in range(MT):
                        lhs = sr_all[:, mt * 128 : (mt + 1) * 128]
                        for h in range(2):
                            nc.tensor.matmul(
                                ps_out[:, h * 512 : (h + 1) * 512],
                                lhs,
                                wv[:, mt * NCH + h * 512 : mt * NCH + (h + 1) * 512],
                                start=(mt == 0),
                                stop=(mt == MT - 1),
                            )
                        if mt == MT // 2 - 1:
                            filler(FILL_D)
                    filler(FILL_D)
                    ot = opool.tile([128, NCH], SDT, tag="ot", bufs=3)
                    nc.vector.tensor_copy(ot, ps_out)
                    nc.gpsimd.dma_start(
                        out=outp.ap()[c * C_OUT : (c + 1) * C_OUT, :], in_=ot
                    )
                else:
                    # h-outer on the final chunk: the first half's cast+flush
                    # overlaps the second half's matmuls, shortening the tail
                    for h in range(2):
                        for mt in range(MT):
                            lhs = sr_all[:, mt * 128 : (mt + 1) * 128]
                            nc.tensor.matmul(
                                ps_out[:, h * 512 : (h + 1) * 512],
                                lhs,
                                wv[:, mt * NCH + h * 512 : mt * NCH + (h + 1) * 512],
                                start=(mt == 0),
                                stop=(mt == MT - 1),
                            )
                        ot = opool.tile([128, 512], SDT, tag="oth", bufs=2)
                        nc.vector.tensor_copy(ot, ps_out[:, h * 512 : (h + 1) * 512])
                        nc.gpsimd.dma_start(
                            out=outp.ap()[
                                c * C_OUT : (c + 1) * C_OUT,
                                h * 512 : (h + 1) * 512,
                            ],
                            in_=ot,
                        )
    nc.compile()
    return nc


def make_in_maps(features, wavelets, wavelets_inv, weight_matrix, filt):
    import ml_dtypes

    sdt = ml_dtypes.bfloat16
    features = np.ascontiguousarray(features, dtype=np.float32)
    wavelets = np.ascontiguousarray(wavelets, dtype=np.float32)
    wavelets_inv = np.ascontiguousarray(wavelets_inv, dtype=np.float32)
    weight_matrix = np.ascontiguousarray(weight_matrix, dtype=np.float32)
    filt = np.ascontiguousarray(filt, dtype=np.float32)

    t_full = features @ weight_matrix
    # row p*KT+g holds T[g*128+p]: per-partition-contiguous for 16KB descriptors
    t_host = np.ascontiguousarray(
        t_full.reshape(KT, 128, C_OUT).transpose(1, 0, 2).reshape(N, C_OUT)
    ).astype(sdt)
    in_maps = []
    for j in range(M):
        jb = slice(j * B, (j + 1) * B)
        wt = (wavelets_inv[jb, :] * filt[jb, None]).T  # [N, B]
        # within each 2MB group: row p*GK+a holds wt[g*1024 + a*128 + p]
        winv_t = np.ascontiguousarray(
            wt.reshape(NG, GK, 128, B).transpose(0, 2, 1, 3).reshape(N, B)
        ).astype(sdt)
        # chunk-major blocking of wavelets[:, jb].T: row c*B + m
        wav_t = wavelets[:, jb].T  # [B, N]
        # chunk-major; within a chunk: row p*MT+mt holds wav_t[mt*128+p]
        wav_b = np.ascontiguousarray(
            wav_t.reshape(MT, 128, NC, NCH).transpose(2, 1, 0, 3).reshape(NC * B, NCH)
        ).astype(sdt)
        in_maps.append(
            {
                "t_d": t_host,
                "winv_t": winv_t,
                "wav_b": wav_b,
                "ident": np.eye(128, dtype=np.float32).astype(sdt),
            }
        )
    return in_maps


def combine_outputs(results):
    acc = results[0]["outp"].astype(np.float64)
    for j in range(1, M):
        acc += results[j]["outp"]
    # outp rows are [c][cc]: row c*C_OUT + cc holds out^T[cc, c*NCH:...]
    out_t = acc.reshape(NC, C_OUT, NCH).transpose(1, 0, 2).reshape(C_OUT, N)
    return np.ascontiguousarray(out_t.T.astype(np.float32))


def kernel(features, wavelets, wavelets_inv, weight_matrix, filt):
    os.environ.setdefault("BASS_NEVER_TRACE", "1")
    if "nc" not in _cache:
        _cache["nc"] = _build()
    nc = _cache["nc"]
    in_maps = make_in_maps(features, wavelets, wavelets_inv, weight_matrix, filt)
    res = run_bass_kernel_spmd(nc, in_maps, core_ids=list(range(M)))
    return combine_outputs(res.results)
